# revision 2
# baseline (speedup 1.0000x reference)
"""CausalFFTConv Trainium2 kernel.

y[b,t,d] = sum_{s<=t} x[b,s,d] * k[t-s,d],  k[n,d] = exp(-|decay_d|*n)*cos(freq_d*n)

The kernel k is Re(lam_d^n) with lam_d = exp(-|decay_d| + i*freq_d), so the
convolution is a first-order complex linear recurrence (diagonal SSM):
    H[t] = lam * H[t-1] + x[t],   y[t] = Re(H[t])
computed with a chunked parallel scan instead of an FFT:

  * time is tiled into 128-row blocks (4 chunks of L=32) with channels+batch
    on the free axis [128 t, 512 (b,d)]
  * intra-chunk: u[i] = x[i] * lam^(m-i) (balanced modulation, m=15.5),
    v = blockdiag-lower-tri @ u on the PE (float32r), y = Re(lam^(j-m) * v)
  * inter-chunk: chunk sums are assembled into a [chunk, (b,d)] layout,
    premultiplied by lam^32, PE-transposed to [d, (b,chunk)], scanned with a
    Hillis-Steele doubling scan (scalar_tensor_tensor, per-partition complex
    multipliers), transposed back, and injected into the PE matmuls as a
    broadcast carry via selector matrices.

Work is split across 8 NeuronCores by channel (128 channels each); no
cross-core communication is needed.
"""
import sys

sys.path.insert(0, "/opt/trn_rl_repo")

import numpy as np

B, T, D = 4, 8192, 1024
NCORES = 8
DL = D // NCORES  # 128 channels per core
L = 32  # chunk length
MID = (L - 1) / 2.0  # 15.5, balanced modulation midpoint
NBLK = T // 128  # 64 time blocks of 128 rows
NCB = T // L  # 256 chunks per batch
FREE = 4 * DL  # 512 free = (b, d)

_prog = None  # cached compiled Bass program


def _pow_tables(decay, freq):
    """Host-side per-core constant tables. decay/freq: [DL] float32."""
    dlt = np.abs(decay.astype(np.float64))  # [DL]
    frq = freq.astype(np.float64)

    def pw(z):
        # lam^z per channel: z scalar or [n] -> (re, im) each [n?, DL] f64
        z = np.asarray(z, np.float64)
        mag = np.exp(-dlt[None, :] * z[:, None])
        return mag * np.cos(frq[None, :] * z[:, None]), mag * np.sin(
            frq[None, :] * z[:, None]
        )

    i = np.arange(L, dtype=np.float64)
    pr, pi = pw(MID - i)  # [32, DL]
    qr, qi = pw(i - MID)

    def blk(a):  # [32, DL] -> [128, 512] tiled over 4 chunks (part) x 4 batches
        a128 = np.tile(a, (4, 1))  # [128, DL]
        return np.tile(a128, (1, 4)).astype(np.float32)  # [128, 4*DL] (b major)

    P4r, P4i = blk(pr), blk(pi)
    Q4r, Q4ni = blk(qr), blk(-qi)

    alr, ali = pw(np.array([float(L)]))  # lam^32 [1, DL]
    row = np.tile(alr[0], 4), np.tile(ali[0], 4)  # [(b,d)=512]
    ALr = np.broadcast_to(row[0], (128, FREE)).astype(np.float32).copy()
    ALi = np.broadcast_to(row[1], (128, FREE)).astype(np.float32).copy()

    # doubling multipliers lam^(32*2^k), k=0..7, per channel, [DL, 8]
    ks = np.arange(8)
    a2r = np.empty((DL, 8), np.float64)
    a2i = np.empty((DL, 8), np.float64)
    for k in ks:
        r_, i_ = pw(np.array([32.0 * (1 << k)]))
        a2r[:, k], a2i[:, k] = r_[0], i_[0]
    A2r = a2r.astype(np.float32)
    A2ni = (-a2i).astype(np.float32)
    A2i = a2i.astype(np.float32)

    return {
        "p4r": P4r,
        "p4i": P4i,
        "q4r": Q4r,
        "q4ni": Q4ni,
        "alr": ALr,
        "ali": ALi,
        "a2r": A2r,
        "a2i": A2i,
        "a2ni": A2ni,
    }


def _shared_tables():
    LT4 = np.zeros((128, 128), np.float32)  # lhsT[k,j]=1 iff same chunk, k<=j
    for c in range(4):
        LT4[32 * c : 32 * c + 32, 32 * c : 32 * c + 32] = np.tril(
            np.ones((32, 32), np.float32)
        ).T
    # SEL[k]: M=128 selector, out rows {32k+c} = sum over chunk c, zeros else
    sels = []
    for k in range(4):
        M = np.zeros((128, 128), np.float32)
        for c in range(4):
            M[32 * c : 32 * c + 32, 32 * k + c] = 1.0
        sels.append(M)
    SEL = np.concatenate(sels, axis=1)  # [128, 512]
    # E32[m]: carry bcast lhsT; at any 32-aligned base bp:
    # lhsT[bp+r, j] = 1 iff r == 4m + j//32  -> out[j] = G[bp + 4m + j//32]
    e32s = []
    for m in range(8):
        M = np.zeros((128, 128), np.float32)
        for g in range(4):
            for c in range(4):
                M[32 * g + 4 * m + c, 32 * c : 32 * c + 32] = 1.0
        e32s.append(M)
    E32 = np.concatenate(e32s, axis=1)  # [128, 1024]
    IDENT = np.eye(128, dtype=np.float32)
    return {"lt4": LT4, "sel": SEL, "e32": E32, "ident": IDENT}


def _build():
    import concourse.bacc as bacc
    import concourse.mybir as mybir

    F32 = mybir.dt.float32
    F32R = mybir.dt.float32r
    MUL = mybir.AluOpType.mult
    ADD = mybir.AluOpType.add
    SUB = mybir.AluOpType.subtract
    from concourse.tile import TileContext

    nc = bacc.Bacc("TRN2", target_bir_lowering=False, debug=False)

    x_d = nc.dram_tensor("x", [B, T, DL], F32, kind="ExternalInput").ap()
    y_d = nc.dram_tensor("y", [B, T, DL], F32, kind="ExternalOutput").ap()
    cd = {}
    for name, shape in [
        ("p4r", [128, FREE]),
        ("p4i", [128, FREE]),
        ("q4r", [128, FREE]),
        ("q4ni", [128, FREE]),
        ("alr", [128, FREE]),
        ("ali", [128, FREE]),
        ("a2r", [DL, 8]),
        ("a2i", [DL, 8]),
        ("a2ni", [DL, 8]),
        ("lt4", [128, 128]),
        ("sel", [128, 512]),
        ("e32", [128, 1024]),
        ("ident", [128, 128]),
    ]:
        cd[name] = nc.dram_tensor(name, shape, F32, kind="ExternalInput").ap()

    xv = x_d.rearrange("b (blk p) d -> blk p b d", p=128)  # [64,128,4,DL]
    yv = y_d.rearrange("b (blk p) d -> blk p b d", p=128)

    with TileContext(nc) as tc:
        with tc.tile_pool(name="const", bufs=1) as cp:
            ct = {}
            for name in cd:
                shape = cd[name].shape
                t = cp.tile(list(shape), F32, name=f"c_{name}")
                nc.sync.dma_start(out=t[:], in_=cd[name])
                ct[name] = t
            # f32r copies of matmul weights (DVE-produced so PE deps stay simple)
            lt4r = cp.tile([128, 128], F32R, name="lt4r")
            nc.vector.tensor_copy(lt4r[:], ct["lt4"][:])
            selr = cp.tile([128, 512], F32R, name="selr")
            nc.vector.tensor_copy(selr[:], ct["sel"][:])
            e32r = cp.tile([128, 1024], F32R, name="e32r")
            nc.vector.tensor_copy(e32r[:], ct["e32"][:])

            # long-lived buffers
            sst = {}
            for nm in ("sr0", "sr1", "si0", "si1"):
                sst[nm] = cp.tile([128, FREE], F32, name=f"sst_{nm}")
            hb = {}
            for nm in ("hr", "hi", "h2r", "h2i"):
                hb[nm] = cp.tile([128, 4 * NCB], F32, name=f"hb_{nm}")
            gst = {}
            for nm in ("gr0", "gr1", "gi0", "gi1"):
                gst[nm] = cp.tile([128, FREE], F32R, name=f"gst_{nm}")

            # ---------------- phase 1: chunk sums ----------------
            with (
                tc.tile_pool(name="x1", bufs=3) as xp,
                tc.tile_pool(name="u1", bufs=3) as up,
                tc.tile_pool(name="sx", bufs=2) as sxp,
                tc.tile_pool(name="sps", bufs=2, space="PSUM") as spp,
            ):
                for g in range(16):
                    spr = spp.tile([128, FREE], F32, name="spr", tag="spr")
                    spi = spp.tile([128, FREE], F32, name="spi", tag="spi")
                    for k in range(4):
                        beta = 4 * g + k
                        xt = xp.tile([128, FREE], F32, name="xt", tag="xt")
                        nc.sync.dma_start(out=xt[:], in_=xv[beta])
                        ur = up.tile([128, FREE], F32R, name="ur", tag="ur")
                        ui = up.tile([128, FREE], F32R, name="ui", tag="ui")
                        nc.vector.tensor_tensor(ur[:], xt[:], ct["p4r"][:], op=MUL)
                        nc.gpsimd.tensor_tensor(ui[:], xt[:], ct["p4i"][:], op=MUL)
                        nc.tensor.matmul(
                            spr[:],
                            selr[:, 128 * k : 128 * k + 128],
                            ur[:],
                            start=(k == 0),
                            stop=(k == 3),
                        )
                        nc.tensor.matmul(
                            spi[:],
                            selr[:, 128 * k : 128 * k + 128],
                            ui[:],
                            start=(k == 0),
                            stop=(k == 3),
                        )
                    sxr = sxp.tile([128, FREE], F32, name="sxr", tag="sxr")
                    sxi = sxp.tile([128, FREE], F32, name="sxi", tag="sxi")
                    nc.vector.tensor_copy(sxr[:], spr[:])
                    nc.vector.tensor_copy(sxi[:], spi[:])
                    for k in range(4):
                        beta = 4 * g + k
                        r0 = (4 * beta) % 128
                        sr = sst["sr0"] if beta < 32 else sst["sr1"]
                        si = sst["si0"] if beta < 32 else sst["si1"]
                        nc.sync.dma_start(
                            out=sr[r0 : r0 + 4, :], in_=sxr[32 * k : 32 * k + 4, :]
                        )
                        nc.sync.dma_start(
                            out=si[r0 : r0 + 4, :], in_=sxi[32 * k : 32 * k + 4, :]
                        )

            # ---------------- premult by lam^32 (chunk-major layout) --------
            with tc.tile_pool(name="pm", bufs=1) as pmp:
                for ctile in range(2):
                    sr = sst["sr0" if ctile == 0 else "sr1"]
                    si = sst["si0" if ctile == 0 else "si1"]
                    t1 = pmp.tile([128, FREE], F32, name="t1", tag="pm1")
                    t2 = pmp.tile([128, FREE], F32, name="t2", tag="pm2")
                    t3 = pmp.tile([128, FREE], F32, name="t3", tag="pm3")
                    t4 = pmp.tile([128, FREE], F32, name="t4", tag="pm4")
                    nc.vector.tensor_tensor(t1[:], sr[:], ct["alr"][:], op=MUL)
                    nc.gpsimd.tensor_tensor(t2[:], si[:], ct["ali"][:], op=MUL)
                    nc.vector.tensor_tensor(t3[:], sr[:], ct["ali"][:], op=MUL)
                    nc.gpsimd.tensor_tensor(t4[:], si[:], ct["alr"][:], op=MUL)
                    nc.vector.tensor_tensor(sr[:], t1[:], t2[:], op=SUB)
                    nc.vector.tensor_tensor(si[:], t3[:], t4[:], op=ADD)

            # ---------------- transpose S -> Hbuf [d, (b, c)] ---------------
            with tc.tile_pool(name="tp1", bufs=2, space="PSUM") as tpp:
                for ctile in range(2):
                    for b in range(4):
                        for comp in ("r", "i"):
                            src = sst[f"s{comp}{ctile}"]
                            dst = hb["hr" if comp == "r" else "hi"]
                            tp = tpp.tile([128, 128], F32, name="tp", tag="tp")
                            nc.tensor.transpose(
                                tp[:],
                                src[:, 128 * b : 128 * b + 128],
                                ct["ident"][:],
                            )
                            nc.vector.tensor_copy(
                                dst[:, 256 * b + 128 * ctile : 256 * b + 128 * ctile + 128],
                                tp[:],
                            )

            # ---------------- doubling scan over chunks ---------------------
            cur_r, cur_i = hb["hr"], hb["hi"]
            nxt_r, nxt_i = hb["h2r"], hb["h2i"]
            c3 = lambda t: t[:].rearrange("p (b c) -> p b c", b=4)
            for k in range(8):
                s = 1 << k
                cr, ci, nr, ni = c3(cur_r), c3(cur_i), c3(nxt_r), c3(nxt_i)
                nc.vector.tensor_copy(nr[:, :, 0:s], cr[:, :, 0:s])
                nc.vector.tensor_copy(ni[:, :, 0:s], ci[:, :, 0:s])
                nc.vector.scalar_tensor_tensor(
                    nr[:, :, s:NCB],
                    cr[:, :, 0 : NCB - s],
                    ct["a2r"][:, k : k + 1],
                    cr[:, :, s:NCB],
                    op0=MUL,
                    op1=ADD,
                )
                nc.vector.scalar_tensor_tensor(
                    nr[:, :, s:NCB],
                    ci[:, :, 0 : NCB - s],
                    ct["a2ni"][:, k : k + 1],
                    nr[:, :, s:NCB],
                    op0=MUL,
                    op1=ADD,
                )
                nc.vector.scalar_tensor_tensor(
                    ni[:, :, s:NCB],
                    ci[:, :, 0 : NCB - s],
                    ct["a2r"][:, k : k + 1],
                    ci[:, :, s:NCB],
                    op0=MUL,
                    op1=ADD,
                )
                nc.vector.scalar_tensor_tensor(
                    ni[:, :, s:NCB],
                    cr[:, :, 0 : NCB - s],
                    ct["a2i"][:, k : k + 1],
                    ni[:, :, s:NCB],
                    op0=MUL,
                    op1=ADD,
                )
                cur_r, nxt_r = nxt_r, cur_r
                cur_i, nxt_i = nxt_i, cur_i
            # G = exclusive shift of J (scan result) per batch
            g_r, g_i = nxt_r, nxt_i  # reuse the other ping-pong buffers
            jr, ji, gr3, gi3 = c3(cur_r), c3(cur_i), c3(g_r), c3(g_i)
            nc.vector.memset(gr3[:, :, 0:1], 0.0)
            nc.vector.memset(gi3[:, :, 0:1], 0.0)
            nc.vector.tensor_copy(gr3[:, :, 1:NCB], jr[:, :, 0 : NCB - 1])
            nc.vector.tensor_copy(gi3[:, :, 1:NCB], ji[:, :, 0 : NCB - 1])

            # ---------------- transpose back G -> Gstage [c, (b, d)] --------
            with tc.tile_pool(name="tp2", bufs=2, space="PSUM") as tpp:
                for ctile in range(2):
                    for b in range(4):
                        for comp in ("r", "i"):
                            src = g_r if comp == "r" else g_i
                            dst = gst[f"g{comp}{ctile}"]
                            tp = tpp.tile([128, 128], F32, name="tpb", tag="tpb")
                            nc.tensor.transpose(
                                tp[:],
                                src[:, 256 * b + 128 * ctile : 256 * b + 128 * ctile + 128],
                                ct["ident"][:],
                            )
                            nc.vector.tensor_copy(
                                dst[:, 128 * b : 128 * b + 128], tp[:]
                            )

            # ---------------- phase 2: full pipeline ------------------------
            with (
                tc.tile_pool(name="x2", bufs=3) as xp,
                tc.tile_pool(name="u2", bufs=3) as up,
                tc.tile_pool(name="t2p", bufs=3) as tp2,
                tc.tile_pool(name="y2", bufs=3) as yp,
                tc.tile_pool(name="vps", bufs=2, space="PSUM") as vpp,
            ):
                for beta in range(NBLK):
                    xt = xp.tile([128, FREE], F32, name="x2t", tag="x2t")
                    nc.sync.dma_start(out=xt[:], in_=xv[beta])
                    ur = up.tile([128, FREE], F32R, name="u2r", tag="u2r")
                    ui = up.tile([128, FREE], F32R, name="u2i", tag="u2i")
                    nc.vector.tensor_tensor(ur[:], xt[:], ct["p4r"][:], op=MUL)
                    nc.gpsimd.tensor_tensor(ui[:], xt[:], ct["p4i"][:], op=MUL)
                    m8 = beta % 8
                    bp = ((beta // 8) % 4) * 32
                    ctile = beta // 32
                    vr = vpp.tile([128, FREE], F32, name="vr", tag="vr")
                    vi = vpp.tile([128, FREE], F32, name="vi", tag="vi")
                    nc.tensor.matmul(vr[:], lt4r[:], ur[:], start=True, stop=False)
                    nc.tensor.matmul(
                        vr[:],
                        e32r[bp : bp + 32, 128 * m8 : 128 * m8 + 128],
                        gst[f"gr{ctile}"][bp : bp + 32, :],
                        start=False,
                        stop=True,
                        tile_position=(bp, 0),
                    )
                    nc.tensor.matmul(vi[:], lt4r[:], ui[:], start=True, stop=False)
                    nc.tensor.matmul(
                        vi[:],
                        e32r[bp : bp + 32, 128 * m8 : 128 * m8 + 128],
                        gst[f"gi{ctile}"][bp : bp + 32, :],
                        start=False,
                        stop=True,
                        tile_position=(bp, 0),
                    )
                    t1 = tp2.tile([128, FREE], F32, name="t1p", tag="t1p")
                    t2 = tp2.tile([128, FREE], F32, name="t2p", tag="t2p")
                    nc.vector.tensor_tensor(t1[:], vr[:], ct["q4r"][:], op=MUL)
                    nc.vector.tensor_tensor(t2[:], vi[:], ct["q4ni"][:], op=MUL)
                    yt = yp.tile([128, FREE], F32, name="yt", tag="yt")
                    nc.gpsimd.tensor_tensor(yt[:], t1[:], t2[:], op=ADD)
                    nc.sync.dma_start(out=yv[beta], in_=yt[:])

    nc.compile()
    return nc


def _get_prog():
    global _prog
    if _prog is None:
        _prog = _build()
    return _prog


def kernel(x, decay, freq):
    from concourse.bass_utils import run_bass_kernel_spmd

    x = np.asarray(x, np.float32)
    decay = np.asarray(decay, np.float32)
    freq = np.asarray(freq, np.float32)
    nc = _get_prog()
    shared = _shared_tables()
    in_maps = []
    for c in range(NCORES):
        sl = slice(c * DL, (c + 1) * DL)
        m = {"x": np.ascontiguousarray(x[:, :, sl])}
        m.update(_pow_tables(decay[sl], freq[sl]))
        m.update(shared)
        in_maps.append(m)
    res = run_bass_kernel_spmd(nc, in_maps, list(range(NCORES)))
    y = np.concatenate([res.results[c]["y"] for c in range(NCORES)], axis=2)
    return y.astype(np.float32)


if __name__ == "__main__":
    rng = np.random.default_rng(0)
    x = rng.standard_normal((B, T, DL * NCORES), dtype=np.float32)
    decay = rng.standard_normal((D,), dtype=np.float32)
    freq = rng.standard_normal((D,), dtype=np.float32)
    y = kernel(x, decay, freq)
    print("y", y.shape, y.dtype)
